# revision 5
# baseline (speedup 1.0000x reference)
"""Trainium2 Bass kernel for nn_Attention_5755256176729.

See reference model in the problem statement: a stateful recurrence over
S=1024 timesteps with B=16, D=E=1024, LayerNorm each step, plus local
(every 16 steps) and global (every 64 steps) summary-state updates, with
input/output linear projections.

Strategy:
  - Host-side folding (float64): the LN affine (ln_w/ln_b) is folded into
    every consumer matmul; chained matmuls are folded:
       W_x   = W_e2s.T @ W_gii          (input proj directly to state space)
       W_gl  = W_gos' @ W_lsi           (gs -> local-summary input, skips out)
       W_gg  = W_gos' @ W_ssi           (gs -> global-summary input)
       W_ll  = W_los' @ W_lsc           (a_u -> next local upre)
       W_sg  = W_sos' @ W_ssc           (a_ug -> next global upre)
       W_out = W_gos' @ W_s2o.T         (gs directly to final output)
    so `out_t` is never materialized and only plain LN0 (no affine) is
    needed on device. Requires ln_b == 0 / b_e2s == 0 / b_s2o == 0 for the
    constant terms to vanish (true for this problem; asserted on host).
  - All matmuls run in float32r (fp32 storage, ~FP22 in the PE, full rate
    at moving-dim >= 256). End-to-end accuracy ~2e-4.
  - The time recurrence is sequential and per-step cost is dominated by
    streaming W_gsc (1M elements) through the PE (~3.4us/step floor,
    batch-independent), so parallelism can't help the recurrence without
    per-step collectives (floor 5-10us >> step). Instead all 8 cores run
    the recurrence REDUNDANTLY and shard the parallel pre/post work:
      phase A: each core computes xs for its 128-timestep slice
      AllGather -> every core has the full xs
      phase B: redundant recurrence; state kept transposed (aT [D,16]
               tiles) as the stationary operand; LN via bn_stats/bn_aggr;
               per-step PE transpose produces the next aT directly.
      phase C: each core computes final = a @ W_out for its own 128
               timesteps (dynamic offsets from the partition id).
  - The six boundary-only weight matrices are streamed from HBM each use
    window (all-fp32 doesn't fit SBUF); W_gsc stays resident.
"""
import contextlib
import os
import sys

import numpy as np

sys.path.insert(0, '/opt/trn_rl_repo')

B, S_FULL, E, D = 16, 1024, 1024, 1024
GLOBAL_FREQ, LOCAL_FREQ = 64, 16
LN_EPS = 1e-5
N_CORES = 8

_cache = {}


def _build(S):
    import concourse.bass as bass
    import concourse.mybir as mybir
    import concourse.tile as tile
    from concourse import bacc

    f32 = mybir.dt.float32
    f32r = mybir.dt.float32r
    Ssh = S // N_CORES          # timesteps per core in phases A/C
    NG = Ssh // 8               # groups of 8 timesteps per core
    KT = 8                      # 128-row contraction tiles
    H = 512                     # psum half width

    nc = bacc.Bacc("TRN2")

    embT_d = nc.dram_tensor("embT", [KT, 128, Ssh, B], f32r, kind="ExternalInput")
    wx_d = nc.dram_tensor("wx", [KT, 128, D], f32r, kind="ExternalInput")
    wgsc_d = nc.dram_tensor("wgsc", [KT, 128, D], f32r, kind="ExternalInput")
    wgl_d = nc.dram_tensor("wgl", [KT, 128, D], f32r, kind="ExternalInput")
    wll_d = nc.dram_tensor("wll", [KT, 128, D], f32r, kind="ExternalInput")
    wlos_d = nc.dram_tensor("wlos", [KT, 128, D], f32r, kind="ExternalInput")
    wgg_d = nc.dram_tensor("wgg", [KT, 128, D], f32r, kind="ExternalInput")
    wsg_d = nc.dram_tensor("wsg", [KT, 128, D], f32r, kind="ExternalInput")
    wsos_d = nc.dram_tensor("wsos", [KT, 128, D], f32r, kind="ExternalInput")
    wout_d = nc.dram_tensor("wout", [KT, 128, E], f32r, kind="ExternalInput")
    ident16_d = nc.dram_tensor("ident16", [16, 16], f32r, kind="ExternalInput")
    inj48_d = nc.dram_tensor("inj48", [48, 16], f32r, kind="ExternalInput")

    final_d = nc.dram_tensor("final", [B, Ssh, E], f32, kind="ExternalOutput")

    xw_my_d = nc.dram_tensor("xw_my", [Ssh, B, D], f32r, kind="Internal")
    xw_d = nc.dram_tensor("xw_full", [S, B, D], f32r, kind="Internal",
                          addr_space="Shared")
    aT_d = nc.dram_tensor("aT_buf", [KT, 128, S, B], f32r, kind="Internal")

    with tile.TileContext(nc) as tc, contextlib.ExitStack() as ctx:
        consts = ctx.enter_context(tc.tile_pool(name="consts", bufs=1))
        i16 = consts.tile([16, 16], f32r, tag="i16")
        i48 = consts.tile([48, 16], f32r, tag="i48")
        eps_t = consts.tile([16, 1], f32, tag="eps")
        nc.sync.dma_start(i16[:], ident16_d[:])
        nc.sync.dma_start(i48[:], inj48_d[:])
        nc.vector.memset(eps_t[:], LN_EPS)

        bigw = ctx.enter_context(tc.tile_pool(name="bigw", bufs=1))
        wstream = ctx.enter_context(tc.tile_pool(name="wstream", bufs=4))

        # ================= PHASE A: xw = embT.T @ W_x ====================
        with (
            tc.tile_pool(name="pa_sb", bufs=3) as pa_sb,
            tc.tile_pool(name="pa_ps", bufs=4, space="PSUM") as pa_ps,
            tc.tile_pool(name="pa_out", bufs=3) as pa_out,
        ):
            wx = bigw.tile([128, KT, D], f32r, tag="big")
            for k in range(KT):
                nc.sync.dma_start(wx[:, k, :], wx_d[k])
            for g in range(NG):
                lhs = pa_sb.tile([128, KT, 8, B], f32r, tag="pa_lhs")
                for k in range(KT):
                    nc.sync.dma_start(
                        lhs[:, k, :, :], embT_d[k, :, g * 8:(g + 1) * 8, :])
                xo = pa_out.tile([128, D], f32r, tag="pa_xo")
                for h in range(2):
                    zx = pa_ps.tile([128, H], f32, tag="pa_z")
                    for k in range(KT):
                        nc.tensor.matmul(
                            zx[:],
                            lhs[:, k, :, :].rearrange("p a b -> p (a b)"),
                            wx[:, k, h * H:(h + 1) * H],
                            start=(k == 0), stop=(k == KT - 1))
                    if h == 0:
                        nc.scalar.copy(xo[:, h * H:(h + 1) * H], zx[:])
                    else:
                        nc.vector.tensor_copy(xo[:, h * H:(h + 1) * H], zx[:])
                nc.sync.dma_start(
                    xw_my_d[g * 8:(g + 1) * 8].rearrange("t b d -> (t b) d"),
                    xo[:])

        tc.strict_bb_all_engine_barrier()
        nc.gpsimd.collective_compute(
            "AllGather",
            mybir.AluOpType.bypass,
            ins=[xw_my_d[:].opt()],
            outs=[xw_d[:].opt()],
            replica_groups=[list(range(N_CORES))],
        )
        tc.strict_bb_all_engine_barrier()

        # ================= PHASE B: the recurrence =======================
        with (
            tc.tile_pool(name="pb_state", bufs=1) as pb_state,
            tc.tile_pool(name="pb_a", bufs=2) as pb_a,
            tc.tile_pool(name="pb_auT", bufs=2) as pb_auT,
            tc.tile_pool(name="pb_st", bufs=3) as pb_st,
            tc.tile_pool(name="pb_z", bufs=1, space="PSUM") as pb_z,
            tc.tile_pool(name="pb_t", bufs=2, space="PSUM") as pb_t,
            tc.tile_pool(name="pb_bnd", bufs=2, space="PSUM") as pb_bnd,
        ):
            aT = [pb_state.tile([128, KT, B], f32r, tag=f"aT{i}", name=f"aT{i}")
                  for i in range(2)]
            sxw = [pb_state.tile([48, D], f32r, tag=f"sxw{i}", name=f"sxw{i}")
                   for i in range(2)]
            upre = pb_state.tile([16, D], f32r, tag="upre")
            upre_g = pb_state.tile([16, D], f32r, tag="upre_g")
            lss_sb = pb_state.tile([16, D], f32, tag="lss")
            gss_sb = pb_state.tile([16, D], f32, tag="gss")
            for i in range(2):
                nc.vector.memset(aT[i][:].bitcast(f32), 0.0)
                nc.vector.memset(sxw[i][32:48, :].bitcast(f32), 0.0)
            nc.vector.memset(upre[:].bitcast(f32), 0.0)
            nc.vector.memset(upre_g[:].bitcast(f32), 0.0)
            nc.vector.memset(lss_sb[:], 0.0)
            nc.vector.memset(gss_sb[:], 0.0)

            wgsc = bigw.tile([128, KT, D], f32r, tag="big")
            for k in range(KT):
                nc.sync.dma_start(wgsc[:, k, :], wgsc_d[k])

            def ln_chain(z_ps, a_out):
                """a_out = (z - mean(z)) * rsqrt(var(z) + eps), rowwise."""
                st = pb_st.tile([16, 2, 6], f32, tag="bnst")
                nc.vector.bn_stats(st[:, 0, :], z_ps[:, 0:H])
                nc.vector.bn_stats(st[:, 1, :], z_ps[:, H:2 * H])
                mv = pb_st.tile([16, 2], f32, tag="mv")
                nc.vector.bn_aggr(mv[:], st[:])
                sd = pb_st.tile([16, 1], f32, tag="sd")
                nc.scalar.activation(sd[:], mv[:, 1:2],
                                     mybir.ActivationFunctionType.Sqrt,
                                     bias=eps_t[:], scale=1.0)
                nc.vector.reciprocal(sd[:], sd[:])
                nc.vector.tensor_scalar(
                    out=a_out, in0=z_ps[:], scalar1=mv[:, 0:1], scalar2=sd[:],
                    op0=mybir.AluOpType.subtract, op1=mybir.AluOpType.mult)

            def transpose16(a_sb, dst):
                """a_sb [16, 1024] -> dst [128, KT, 16] via PE transposes."""
                tp = pb_t.tile([128, KT, B], f32r, tag="tps")
                for j in range(KT):
                    nc.tensor.transpose(
                        tp[:, j, :], a_sb[:, j * 128:(j + 1) * 128], i16[:])
                nc.vector.tensor_copy(dst[:], tp[:])

            def mm_1024(dst_ps, lhsT_tiles, w_tile, inject_rhs=None,
                        inj_lhs=None):
                """dst_ps[16,1024] = sum_k lhsT[:,k,:].T @ w[:,k,:] (+ inject)."""
                for h in range(2):
                    hs = slice(h * H, (h + 1) * H)
                    for k in range(KT):
                        nc.tensor.matmul(
                            dst_ps[:, hs], lhsT_tiles[:, k, :], w_tile[:, k, hs],
                            start=(k == 0),
                            stop=(inject_rhs is None and k == KT - 1))
                    if inject_rhs is not None:
                        nc.tensor.matmul(dst_ps[:, hs], inj_lhs,
                                         inject_rhs[:, hs],
                                         start=False, stop=True)

            stream_tiles = {}

            def stream_in(name, dram):
                t_ = wstream.tile([128, KT, D], f32r, tag="ws")
                for k in range(KT):
                    nc.sync.dma_start(t_[:, k, :], dram[k])
                stream_tiles[name] = t_

            auT_prev = [None]
            augT_prev = [None]

            for t in range(S):
                cur, nxt = t % 2, (t + 1) % 2
                seg = t // LOCAL_FREQ
                win = t // GLOBAL_FREQ
                # ---- weight streaming schedule ----
                if t % LOCAL_FREQ == 0:
                    stream_in('wgl', wgl_d)
                    stream_in('wlos', wlos_d)
                if t % LOCAL_FREQ == 1 and seg > 0:
                    stream_in('wll', wll_d)
                if t % GLOBAL_FREQ == 16 and win > 0:
                    stream_in('wsg', wsg_d)
                if t % GLOBAL_FREQ == 56:
                    stream_in('wgg', wgg_d)
                    stream_in('wsos', wsos_d)

                # ---- deferred upre computations ----
                if t % LOCAL_FREQ == 6 and seg > 0:
                    up_ps = pb_bnd.tile([16, D], f32, tag="bnd")
                    mm_1024(up_ps, auT_prev[0], stream_tiles['wll'])
                    nc.scalar.copy(upre[:], up_ps[:])
                if t % GLOBAL_FREQ == 22 and win > 0:
                    upg_ps = pb_bnd.tile([16, D], f32, tag="bnd")
                    mm_1024(upg_ps, augT_prev[0], stream_tiles['wsg'])
                    nc.scalar.copy(upre_g[:], upg_ps[:])

                # ---- main step ----
                nc.sync.dma_start(sxw[cur][0:16, :], xw_d[t])
                z = pb_z.tile([16, D], f32, tag="z")
                mm_1024(z, aT[cur], wgsc, inject_rhs=sxw[cur], inj_lhs=i48[:])
                a = pb_a.tile([16, D], f32r, tag="a")
                ln_chain(z, a[:])
                transpose16(a, aT[nxt])
                nc.sync.dma_start(
                    aT_d[:, :, t, :].rearrange("k p b -> p k b"), aT[nxt][:])

                # ---- boundary work ----
                if t % LOCAL_FREQ == LOCAL_FREQ - 1:
                    u_ps = pb_bnd.tile([16, D], f32, tag="bnd")
                    mm_1024(u_ps, aT[nxt], stream_tiles['wgl'],
                            inject_rhs=upre, inj_lhs=i16[:])
                    a_u = pb_a.tile([16, D], f32r, tag="au")
                    ln_chain(u_ps, a_u[:])
                    auT = pb_auT.tile([128, KT, B], f32r, tag="auT")
                    transpose16(a_u, auT)
                    auT_prev[0] = auT
                    lp = pb_bnd.tile([16, D], f32, tag="bnd")
                    mm_1024(lp, auT, stream_tiles['wlos'])
                    nc.scalar.copy(lss_sb[:], lp[:])

                    if t % GLOBAL_FREQ == GLOBAL_FREQ - 1:
                        ug_ps = pb_bnd.tile([16, D], f32, tag="bnd")
                        mm_1024(ug_ps, aT[nxt], stream_tiles['wgg'],
                                inject_rhs=upre_g, inj_lhs=i16[:])
                        a_ug = pb_a.tile([16, D], f32r, tag="au")
                        ln_chain(ug_ps, a_ug[:])
                        augT = pb_auT.tile([128, KT, B], f32r, tag="augT")
                        transpose16(a_ug, augT)
                        augT_prev[0] = augT
                        gp = pb_bnd.tile([16, D], f32, tag="bnd")
                        mm_1024(gp, augT, stream_tiles['wsos'])
                        nc.scalar.copy(gss_sb[:], gp[:])

                    nc.vector.tensor_add(sxw[nxt][32:48, :], gss_sb[:],
                                         lss_sb[:])
                    nc.vector.tensor_copy(sxw[cur][32:48, :],
                                          sxw[nxt][32:48, :])

        tc.strict_bb_all_engine_barrier()

        # ================= PHASE C: final = a @ W_out ====================
        with (
            tc.tile_pool(name="pc_sb", bufs=3) as pc_sb,
            tc.tile_pool(name="pc_ps", bufs=4, space="PSUM") as pc_ps,
            tc.tile_pool(name="pc_out", bufs=3) as pc_out,
        ):
            wout = wstream.tile([128, KT, E], f32r, tag="ws")
            for k in range(KT):
                nc.sync.dma_start(wout[:, k, :], wout_d[k])
            pid = nc.sync.partition_id()
            t_base = pid * Ssh
            for g in range(NG):
                lhs = pc_sb.tile([128, KT, 8, B], f32r, tag="pc_lhs")
                for k in range(KT):
                    nc.sync.dma_start(
                        lhs[:, k, :, :],
                        aT_d[k, :, bass.ds(t_base + g * 8, 8), :])
                fo = pc_out.tile([128, E], f32, tag="pc_fo")
                for h in range(2):
                    zc = pc_ps.tile([128, H], f32, tag="pc_z")
                    for k in range(KT):
                        nc.tensor.matmul(
                            zc[:],
                            lhs[:, k, :, :].rearrange("p a b -> p (a b)"),
                            wout[:, k, h * H:(h + 1) * H],
                            start=(k == 0), stop=(k == KT - 1))
                    if h == 0:
                        nc.scalar.copy(fo[:, h * H:(h + 1) * H], zc[:])
                    else:
                        nc.vector.tensor_copy(fo[:, h * H:(h + 1) * H], zc[:])
                for tl in range(8):
                    nc.sync.dma_start(
                        final_d[:, g * 8 + tl, :],
                        fo[tl * B:(tl + 1) * B, :])

    nc.finalize()
    return nc


def _prepare(inputs, S):
    """Host-side folding (float64) + per-core input maps."""
    f8 = np.float64
    emb = np.asarray(inputs['embedded_tokens'], np.float32)
    ln_w = np.asarray(inputs['ln_w'], f8)
    ln_b = np.asarray(inputs['ln_b'], f8)
    b_e2s = np.asarray(inputs['b_e2s'], f8)
    b_s2o = np.asarray(inputs['b_s2o'], f8)
    assert np.allclose(ln_b, 0) and np.allclose(b_e2s, 0) and \
        np.allclose(b_s2o, 0), "constant folding assumes zero biases"

    W = {k: np.asarray(inputs[k], f8) for k in
         ('W_gsc', 'W_gii', 'W_gos', 'W_lsc', 'W_lsi', 'W_los',
          'W_ssc', 'W_ssi', 'W_sos', 'W_e2s', 'W_s2o')}
    dw = np.diag(ln_w)
    Wp_gsc = dw @ W['W_gsc']
    Wp_gos = dw @ W['W_gos']
    Wp_los = dw @ W['W_los']
    Wp_sos = dw @ W['W_sos']

    mats = {
        'wx': W['W_e2s'].T @ W['W_gii'],
        'wgsc': Wp_gsc,
        'wgl': Wp_gos @ W['W_lsi'],
        'wll': Wp_los @ W['W_lsc'],
        'wlos': Wp_los,
        'wgg': Wp_gos @ W['W_ssi'],
        'wsg': Wp_sos @ W['W_ssc'],
        'wsos': Wp_sos,
        'wout': Wp_gos @ W['W_s2o'].T,
    }
    base = {k: np.ascontiguousarray(
        v.astype(np.float32).reshape(8, 128, -1)) for k, v in mats.items()}
    base['ident16'] = np.eye(16, dtype=np.float32)
    base['inj48'] = np.concatenate(
        [np.eye(16), np.zeros((16, 16)), np.eye(16)]).astype(np.float32)

    Ssh = S // N_CORES
    in_maps = []
    for c in range(N_CORES):
        m = dict(base)
        sh = emb[:, c * Ssh:(c + 1) * Ssh, :]               # [B, Ssh, E]
        embT = np.ascontiguousarray(sh.transpose(2, 1, 0))  # [E, Ssh, B]
        m['embT'] = embT.reshape(8, 128, Ssh, B)
        in_maps.append(m)
    return in_maps


def kernel(**inputs):
    from concourse.bass_utils import run_bass_kernel_spmd

    S = int(os.environ.get('KERNEL_S', S_FULL))
    key = ('bass', S)
    if key not in _cache:
        _cache[key] = _build(S)
    nc = _cache[key]
    in_maps = _prepare(inputs, S)
    res = run_bass_kernel_spmd(
        nc, in_maps, core_ids=list(range(N_CORES)),
        trace=bool(int(os.environ.get('KERNEL_TRACE', '0'))))
    _cache['last_results'] = res
    Ssh = S // N_CORES
    final = np.empty((B, S, E), np.float32)
    for c in range(N_CORES):
        final[:, c * Ssh:(c + 1) * Ssh, :] = res.results[c]['final']
    return final
